# revision 27
# baseline (speedup 1.0000x reference)
"""MultiLinearUpsampling Trainium2 kernel.

Problem: out[b, t, :] = W[lidx[t]] @ pooled[b, segc[t], :]  (zero for invalid t)
where segc/lidx derive from sorted pooling_indices (ragged segments).

Strategy
--------
Only sum_l N_l unique matvecs exist per batch (N_l = #segments with
len > l; positions past offset L-1 reuse the l=L-1 result).  One SPMD
program on 8 cores: S slots, slot s = one stationary weight plane
(per-core data) applied to C_s activation columns x 8 batches
(per-core data, host-gathered).  A packing DP cuts the 16 linears'
column sets into pieces (<=8 per slot across cores) minimizing
sum(C_s) -- the per-core PE streaming time, which dominates.  Matmuls
run in fp16 (full PE rate); accumulation is fp32 in PSUM; outputs are
written back in fp16 (halves output DMA) and upcast on host.

Schedule details (from baseline trace analysis: PE was 100% busy but
head=17us waiting on serialized input DMAs and tail=12us draining
fp32 outputs):
 - all DRAM tensors are laid out so every DMA is 128 fat contiguous
   rows (few descriptors, near-peak bandwidth, low issue cost);
 - slot 0 is the smallest slot, its inputs are DMA'd per-k-chunk
   (w_k || x_k on the two HWDGE queues) and its compute runs k-outer
   so the first matmul only waits for one (w,x) k-pair (~0.4MB);
 - remaining slots load as one W DMA (scalar) + one X DMA (sync)
   each, emitted in processing order, and run k-inner with PSUM bank
   rotation;
 - outputs go per-chunk on gpsimd; the last slot is second-smallest
   and its final chunk drains per-m-slice so the tail is tiny.
"""

from contextlib import ExitStack

import numpy as np

import concourse.bass as bass  # noqa: F401  (bass types via bacc)
import concourse.mybir as mybir
import concourse.tile as tile
from concourse import bacc
from concourse.bass_utils import run_bass_kernel_spmd

F32 = mybir.dt.float32
F16 = mybir.dt.float16

B = 8          # batch (each core sees all batches)
N = 512        # segments
D = 1024       # D_in == D_out
L = 16         # linears
NCORES = 8
KC = 8         # contraction chunks of 128
MC = 8         # output-dim chunks of 128
CHMAX = 512    # PSUM bank = 512 fp32 -> max streamed columns per chunk


# ---------------------------------------------------------------------------
# packing: choose slot capacities + piece assignment
# ---------------------------------------------------------------------------

def _combos_for(sz, Cs, max_waste=70, max_pieces=5, keep=14):
    """Minimal piece-count vectors over slot capacities covering sz."""
    S = len(Cs)
    out = []

    def rec(i, vec, cap, npc):
        if npc > max_pieces or cap > sz + max_waste:
            return
        if cap >= sz and npc > 0:
            for j in range(len(vec)):
                if vec[j] > 0 and cap - Cs[j] >= sz:
                    return  # not minimal
            out.append((cap - sz, tuple(vec) + (0,) * (S - len(vec))))
            return
        if i == S:
            return
        for n in range(0, 9):
            cap2 = cap + n * Cs[i]
            if n and cap2 > sz + max_waste:
                break
            rec(i + 1, vec + [n], cap2, npc + n)

    rec(0, [], 0, 0)
    out.sort()
    return [v for _, v in out[:keep]]


def _fit(Cs, items, max_waste=70):
    """Assign each item a piece-count vector (sum n_p*C_p >= item),
    <=8 pieces per slot overall.  Returns per-item count vectors or
    None."""
    S = len(Cs)
    all_combos = []
    for sz in items:
        cb = _combos_for(sz, Cs, max_waste)
        if not cb:
            return None
        all_combos.append(cb)
    states = {tuple([0] * S): True}
    back = []
    for combos in all_combos:
        nstates = {}
        bk = {}
        for st in states:
            for cb in combos:
                nst = tuple(st[i] + cb[i] for i in range(S))
                if any(v > 8 for v in nst):
                    continue
                if nst not in nstates:
                    nstates[nst] = True
                    bk[nst] = (st, cb)
        if not nstates:
            return None
        keys = sorted(nstates, key=sum)
        pruned = []
        for k in keys:
            if not any(all(q[i] <= k[i] for i in range(S)) for q in pruned):
                pruned.append(k)
        states = {k: True for k in pruned[:400]}
        back.append({k: bk[k] for k in states})
    st = next(iter(states))
    picks = [None] * len(items)
    for i in range(len(items) - 1, -1, -1):
        st_prev, cb = back[i][st]
        picks[i] = cb
        st = st_prev
    return picks


# capacity vectors found by offline search for the benchmark's N_l
# (feasibility is re-checked against the actual data at runtime)
_CANDIDATE_CS = [
    (136, 120, 86, 77, 31),   # sum 450, 0.98% padding
    (158, 113, 73, 59, 49),   # sum 452, 1.4% padding
    (180, 125, 81, 73),       # sum 459
    (158, 132, 94, 80),       # sum 464
    (214, 170, 110),          # sum 494
]


def _plan(N_l):
    """Return (Cs, slot_map): slot capacities and slot_map[core][s] =
    (l, col_start, cnt) or None."""
    order_l = np.argsort(-np.asarray(N_l), kind="stable")
    items = [int(N_l[i]) for i in order_l if N_l[i] > 0]
    kept_l = [int(i) for i in order_l if N_l[i] > 0]
    if not items:
        return [2], [[None] for _ in range(NCORES)]

    best = None  # (sumC, Cs, picks)
    for Cs in _CANDIDATE_CS:
        if best is not None and sum(Cs) >= best[0]:
            continue
        picks = _fit(Cs, items)
        if picks is not None:
            best = (sum(Cs), list(Cs), picks)

    if best is None:
        # guaranteed-feasible fallback: biggest items unsplit in slot 0,
        # the rest in slot 1
        c1 = items[0]
        c2 = items[8] if len(items) > 8 else 0
        Cs = [c1, c2] if c2 else [c1]
        picks = _fit(Cs, items, max_waste=10**9)
        assert picks is not None
        best = (sum(Cs), Cs, picks)

    _, Cs, picks = best
    S = len(Cs)
    slot_pieces = [[] for _ in range(S)]
    for idx, l in enumerate(kept_l):
        sz = items[idx]
        pos = 0
        for p in range(S):
            for _ in range(picks[idx][p]):
                cnt = min(Cs[p], sz - pos)
                if cnt <= 0:
                    continue
                slot_pieces[p].append((l, pos, cnt))
                pos += cnt
        assert pos >= sz, f"l={l} not covered: {pos}/{sz}"

    slot_map = [[None] * S for _ in range(NCORES)]
    for p in range(S):
        assert len(slot_pieces[p]) <= NCORES, (p, slot_pieces[p])
        for c, piece in enumerate(slot_pieces[p]):
            slot_map[c][p] = piece
    return Cs, slot_map


# ---------------------------------------------------------------------------
# device program
# ---------------------------------------------------------------------------

def _chunks(total):
    n = -(-total // CHMAX)
    base = total // n
    rem = total - base * n
    return [base + (1 if i < rem else 0) for i in range(n)]


def _proc_order(Cs):
    desc = list(np.argsort(Cs)[::-1])
    if len(desc) >= 3:
        # mid-size slot first (small enough that its per-k input stream
        # beats the k-outer sweep), then the largest (second k-outer slot,
        # streamed per-k behind slot0's compute), then the rest descending
        return [desc[2], desc[0], desc[1]] + desc[3:]
    return desc


def _slot_chunks(Cs, s):
    F = B * Cs[s]
    proc = _proc_order(Cs)
    if s in proc[:2]:
        # k-outer slots: big first chunk so early k-rounds are slow
        # enough for the per-k input DMAs; all chunks >=128 cols so
        # ldweights stays hidden under the stream
        first = min(F, 448 if s == proc[0] else CHMAX)
        if F - first >= 128:
            return [first] + _chunks(F - first)
        return [F]
    return _chunks(F)


def _build_program(Cs):
    """Inputs per slot s: x{s} (128, KC*8*C_s) f16, w{s} (128, KC*D) f16.
    Outputs per (slot, chunk): y{s}_{i} (128, MC*ch) f16."""
    nc = bacc.Bacc("TRN2", target_bir_lowering=False, debug=False)
    S = len(Cs)
    # processing order: DESCENDING by capacity.  The first slot runs
    # k-outer with per-k-chunk input DMAs, so it only waits on k0's
    # ~0.6MB and each ~1.4-2.9us k-round covers the next k-chunk's
    # delivery; its long compute then buys DMA time for all later
    # slots' (W-heavy) inputs.  Small slots last also makes the output
    # tail tiny.  (Measured: small-first stalls the PE 17-20us because
    # small slots' 2MB weight loads outpace their compute.)
    proc = _proc_order(Cs)

    def slot_chunks(s):
        return _slot_chunks(Cs, s)

    xs, ws, ys = {}, {}, {}
    for s, C in enumerate(Cs):
        xs[s] = nc.dram_tensor(f"x{s}", (128, KC * B * C), F16, kind="ExternalInput")
        ws[s] = nc.dram_tensor(f"w{s}", (128, KC * D), F16, kind="ExternalInput")
        for i, ch in enumerate(slot_chunks(s)):
            ys[s, i] = nc.dram_tensor(
                f"y{s}_{i}", (128, MC * ch), F16, kind="ExternalOutput"
            )

    with tile.TileContext(nc) as tc, ExitStack() as ctx:
        wpool = ctx.enter_context(tc.tile_pool(name="w", bufs=1))
        xpool = ctx.enter_context(tc.tile_pool(name="x", bufs=1))
        opool = ctx.enter_context(tc.tile_pool(name="o", bufs=3))
        ppool = ctx.enter_context(tc.tile_pool(name="ps", bufs=8, space="PSUM"))

        wt, xt = {}, {}
        for s in range(S):
            C = Cs[s]
            wt[s] = wpool.tile([128, KC * D], F16, tag=f"w{s}", name=f"w{s}")
            xt[s] = xpool.tile([128, KC * B * C], F16, tag=f"x{s}", name=f"x{s}")

        # input DMAs in processing order, W on scalar || X on sync (the two
        # HWDGE rings).  Each ring runs one DMA at a time with ~2us fixed
        # completion latency (measured), so the critical path wants FEW
        # DMAs; slot proc[0] is split 3 ways ([k0], [k1:4], [k4:8]) so the
        # first matmul waits only on ~0.6MB while later k-chunks amortize
        # the latency over MB-sized transfers.
        for j, s in enumerate(proc[:2]):
            F = B * Cs[s]
            for i, (k0, k1) in enumerate(((0, 1), (1, 2), (2, 4), (4, KC))):
                qa, qb = (
                    (nc.scalar, nc.sync) if (i + j) % 2 == 0 else (nc.sync, nc.scalar)
                )
                qa.dma_start(
                    wt[s][:, k0 * D : k1 * D], ws[s].ap()[:, k0 * D : k1 * D]
                )
                qb.dma_start(
                    xt[s][:, k0 * F : k1 * F], xs[s].ap()[:, k0 * F : k1 * F]
                )
        for s in proc[2:]:
            nc.scalar.dma_start(wt[s][:], ws[s].ap()[:])
            nc.sync.dma_start(xt[s][:], xs[s].ap()[:])

        def emit_slot(s, k_outer, fine_tail):
            C = Cs[s]
            F = B * C
            chs = slot_chunks(s)
            off = 0
            for i, ch in enumerate(chs):
                ot = opool.tile([128, MC, ch], F16, tag="o", name=f"o{s}_{i}")
                if k_outer:
                    pss = [
                        ppool.tile([128, ch], F32, tag="ps", name=f"ps{s}_{i}_{m}")
                        for m in range(MC)
                    ]
                    for k in range(KC):
                        for m in range(MC):
                            nc.tensor.matmul(
                                pss[m][:],
                                wt[s][:, k * D + m * 128 : k * D + (m + 1) * 128],
                                xt[s][:, k * F + off : k * F + off + ch],
                                start=(k == 0),
                                stop=(k == KC - 1),
                            )
                    for m in range(MC):
                        nc.vector.tensor_copy(ot[:, m], pss[m][:])
                else:
                    for m in range(MC):
                        ps = ppool.tile([128, ch], F32, tag="ps", name=f"ps{s}_{i}_{m}")
                        for k in range(KC):
                            nc.tensor.matmul(
                                ps[:],
                                wt[s][:, k * D + m * 128 : k * D + (m + 1) * 128],
                                xt[s][:, k * F + off : k * F + off + ch],
                                start=(k == 0),
                                stop=(k == KC - 1),
                            )
                        nc.vector.tensor_copy(ot[:, m], ps[:])
                y = ys[s, i]
                if fine_tail and i == len(chs) - 1:
                    # drain the last chunk via the two idle HWDGE rings
                    # as 2-m-slice DMAs: each piece issues right after
                    # its PSUM copies, pipelining the drain with compute
                    h = 2
                    for mp in range(MC // h):
                        q = nc.sync if mp % 2 == 0 else nc.scalar
                        q.dma_start(
                            y.ap()[:, mp * h * ch : (mp + 1) * h * ch].rearrange(
                                "p (m c) -> p m c", m=h
                            ),
                            ot[:, mp * h : (mp + 1) * h],
                        )
                else:
                    nc.gpsimd.dma_start(
                        y.ap().rearrange("p (m c) -> p m c", m=MC), ot[:]
                    )
                off += ch

        for j, s in enumerate(proc):
            emit_slot(s, k_outer=(j < 2), fine_tail=(j == len(proc) - 1))

    nc.compile()
    return nc


# ---------------------------------------------------------------------------
# host wrapper
# ---------------------------------------------------------------------------

def _segment_structure(idx, T):
    t = np.arange(T)
    seg = np.searchsorted(idx, t, side="left")
    valid = seg < N
    segc = np.clip(seg, 0, N - 1)
    start = np.where(segc > 0, idx[np.maximum(segc - 1, 0)] + 1, 0)
    lidx = np.minimum(t - start, L - 1).astype(np.int64)
    lens = np.bincount(segc[valid], minlength=N)
    return t, seg, valid, segc, lidx, lens


def _install_ntff_hook():
    """Profiling-only: register the axon NTFF profile hook (dev use)."""
    import sys
    import types

    try:
        import antenv

        if "antenv.axon_hooks" not in sys.modules:
            mod = types.ModuleType("antenv.axon_hooks")
            holder = [None]
            mod.set_axon_ntff_profile_hook = lambda h: holder.__setitem__(0, h)
            mod.get_axon_ntff_profile_hook = lambda: holder[0]
            sys.modules["antenv.axon_hooks"] = mod
            antenv.axon_hooks = mod
            from trn_agent_boot.trn_boot import _ntff_profile_via_ctypes

            mod.set_axon_ntff_profile_hook(
                _ntff_profile_via_ctypes("/opt/axon/libaxon_pjrt.so")
            )
    except Exception as e:
        print(f"NTFF hook install failed: {e}")


def kernel(pooled_vectors, W, pooling_indices, target_length, _trace=False):
    pooled = np.asarray(pooled_vectors, dtype=np.float32)
    Wf = np.asarray(W, dtype=np.float32)
    idx = np.asarray(pooling_indices).astype(np.int64)
    T = int(np.asarray(target_length))

    t, seg, valid, segc, lidx, lens = _segment_structure(idx, T)

    order = np.argsort(-lens, kind="stable")
    rank_of_seg = np.empty(N, dtype=np.int64)
    rank_of_seg[order] = np.arange(N)
    N_l = (lens[None, :] > np.arange(L)[:, None]).sum(axis=1)

    Cs, slot_map = _plan(N_l)
    S = len(Cs)

    nc = _build_program(Cs)

    # host-side gathered inputs, fp16
    # Xg: (D, B, N) with columns sorted by segment rank
    Xg = np.ascontiguousarray(pooled.transpose(2, 0, 1)[:, :, order]).astype(
        np.float16
    )
    Xg_k = Xg.reshape(KC, 128, B, N)
    # Wt16[l]: (128, KC*D) with row p, block k = W[l][:, k*128+p]
    Wt16 = np.ascontiguousarray(
        Wf.transpose(2, 0, 1).reshape(KC, 128, L, D).transpose(1, 2, 0, 3)
    ).astype(np.float16)  # (128, L, KC, D)

    in_maps = []
    for c in range(NCORES):
        im = {}
        for s in range(S):
            C = Cs[s]
            xp = np.zeros((128, KC, B, C), dtype=np.float16)
            wp = np.zeros((128, KC * D), dtype=np.float16)
            piece = slot_map[c][s]
            if piece is not None:
                l, c0, cnt = piece
                xp[:, :, :, :cnt] = Xg_k[:, :, :, c0 : c0 + cnt].transpose(1, 0, 2, 3)
                wp[:] = Wt16[:, l].reshape(128, KC * D)
            im[f"x{s}"] = np.ascontiguousarray(xp.reshape(128, KC * B * C))
            im[f"w{s}"] = wp
        in_maps.append(im)

    kwargs = {}
    if _trace:
        _install_ntff_hook()
        kwargs = dict(trace=True)
    res = run_bass_kernel_spmd(nc, in_maps, core_ids=list(range(NCORES)), **kwargs)
    results = res.results

    # per-(l, col-rank) -> (core, slot, j) maps
    maxN = int(N_l.max()) if L else 0
    core_of = np.full((L, max(maxN, 1)), -1, dtype=np.int32)
    slot_of = np.zeros((L, max(maxN, 1)), dtype=np.int32)
    j_of = np.zeros((L, max(maxN, 1)), dtype=np.int32)
    for c in range(NCORES):
        for s in range(S):
            piece = slot_map[c][s]
            if piece is None:
                continue
            l, c0, cnt = piece
            core_of[l, c0 : c0 + cnt] = c
            slot_of[l, c0 : c0 + cnt] = s
            j_of[l, c0 : c0 + cnt] = np.arange(cnt)

    Dout = Wf.shape[1]
    out = np.zeros((B, T, Dout), dtype=np.float32)
    tv = t[valid]
    l_t = lidx[valid]
    r_t = rank_of_seg[segc[valid]]
    ct = core_of[l_t, r_t]
    st = slot_of[l_t, r_t]
    jt = j_of[l_t, r_t]
    assert (ct >= 0).all(), "uncovered (l, col) in assignment"

    for s in range(S):
        sel = st == s
        if not sel.any():
            continue
        C = Cs[s]
        chs = _slot_chunks(Cs, s)
        # per core: concat chunks -> (B*C, 1024), then (B, C, 1024)
        Ys = np.empty((NCORES, B, C, Dout), dtype=np.float16)
        for c in range(NCORES):
            parts = []
            for i, ch in enumerate(chs):
                a = results[c][f"y{s}_{i}"].reshape(128, MC, ch)
                parts.append(a.transpose(2, 1, 0).reshape(ch, Dout))
            Ys[c] = np.concatenate(parts, axis=0).reshape(B, C, Dout)
        out[:, tv[sel], :] = Ys[ct[sel], :, jt[sel]].transpose(1, 0, 2).astype(
            np.float32
        )

    if _trace:
        kernel._last_exec_time_ns = res.exec_time_ns
        kernel._last_results = res
    return out


# revision 29
# speedup vs baseline: 1.0094x; 1.0094x over previous
"""MultiLinearUpsampling Trainium2 kernel.

Problem: out[b, t, :] = W[lidx[t]] @ pooled[b, segc[t], :]  (zero for invalid t)
where segc/lidx derive from sorted pooling_indices (ragged segments).

Strategy
--------
Only sum_l N_l unique matvecs exist per batch (N_l = #segments with
len > l; positions past offset L-1 reuse the l=L-1 result).  One SPMD
program on 8 cores: S slots, slot s = one stationary weight plane
(per-core data) applied to C_s activation columns x 8 batches
(per-core data, host-gathered).  A packing DP cuts the 16 linears'
column sets into pieces (<=8 per slot across cores) minimizing
sum(C_s) -- the per-core PE streaming time, which dominates.  Matmuls
run in fp16 (full PE rate); accumulation is fp32 in PSUM; outputs are
written back in fp16 (halves output DMA) and upcast on host.

Schedule details (from baseline trace analysis: PE was 100% busy but
head=17us waiting on serialized input DMAs and tail=12us draining
fp32 outputs):
 - all DRAM tensors are laid out so every DMA is 128 fat contiguous
   rows (few descriptors, near-peak bandwidth, low issue cost);
 - slot 0 is the smallest slot, its inputs are DMA'd per-k-chunk
   (w_k || x_k on the two HWDGE queues) and its compute runs k-outer
   so the first matmul only waits for one (w,x) k-pair (~0.4MB);
 - remaining slots load as one W DMA (scalar) + one X DMA (sync)
   each, emitted in processing order, and run k-inner with PSUM bank
   rotation;
 - outputs go per-chunk on gpsimd; the last slot is second-smallest
   and its final chunk drains per-m-slice so the tail is tiny.
"""

from contextlib import ExitStack

import numpy as np

import concourse.bass as bass  # noqa: F401  (bass types via bacc)
import concourse.mybir as mybir
import concourse.tile as tile
from concourse import bacc
from concourse.bass_utils import run_bass_kernel_spmd

F32 = mybir.dt.float32
F16 = mybir.dt.float16

B = 8          # batch (each core sees all batches)
N = 512        # segments
D = 1024       # D_in == D_out
L = 16         # linears
NCORES = 8
KC = 8         # contraction chunks of 128
MC = 8         # output-dim chunks of 128
CHMAX = 512    # PSUM bank = 512 fp32 -> max streamed columns per chunk


# ---------------------------------------------------------------------------
# packing: choose slot capacities + piece assignment
# ---------------------------------------------------------------------------

def _combos_for(sz, Cs, max_waste=70, max_pieces=5, keep=14):
    """Minimal piece-count vectors over slot capacities covering sz."""
    S = len(Cs)
    out = []

    def rec(i, vec, cap, npc):
        if npc > max_pieces or cap > sz + max_waste:
            return
        if cap >= sz and npc > 0:
            for j in range(len(vec)):
                if vec[j] > 0 and cap - Cs[j] >= sz:
                    return  # not minimal
            out.append((cap - sz, tuple(vec) + (0,) * (S - len(vec))))
            return
        if i == S:
            return
        for n in range(0, 9):
            cap2 = cap + n * Cs[i]
            if n and cap2 > sz + max_waste:
                break
            rec(i + 1, vec + [n], cap2, npc + n)

    rec(0, [], 0, 0)
    out.sort()
    return [v for _, v in out[:keep]]


def _fit(Cs, items, max_waste=70):
    """Assign each item a piece-count vector (sum n_p*C_p >= item),
    <=8 pieces per slot overall.  Returns per-item count vectors or
    None."""
    S = len(Cs)
    all_combos = []
    for sz in items:
        cb = _combos_for(sz, Cs, max_waste)
        if not cb:
            return None
        all_combos.append(cb)
    states = {tuple([0] * S): True}
    back = []
    for combos in all_combos:
        nstates = {}
        bk = {}
        for st in states:
            for cb in combos:
                nst = tuple(st[i] + cb[i] for i in range(S))
                if any(v > 8 for v in nst):
                    continue
                if nst not in nstates:
                    nstates[nst] = True
                    bk[nst] = (st, cb)
        if not nstates:
            return None
        keys = sorted(nstates, key=sum)
        pruned = []
        for k in keys:
            if not any(all(q[i] <= k[i] for i in range(S)) for q in pruned):
                pruned.append(k)
        states = {k: True for k in pruned[:400]}
        back.append({k: bk[k] for k in states})
    st = next(iter(states))
    picks = [None] * len(items)
    for i in range(len(items) - 1, -1, -1):
        st_prev, cb = back[i][st]
        picks[i] = cb
        st = st_prev
    return picks


# capacity vectors found by offline search for the benchmark's N_l
# (feasibility is re-checked against the actual data at runtime)
_CANDIDATE_CS = [
    (158, 113, 73, 59, 49),   # sum 452, 1.4% padding
    (136, 120, 86, 77, 31),   # sum 450, 0.98% padding
    (180, 125, 81, 73),       # sum 459
    (158, 132, 94, 80),       # sum 464
    (214, 170, 110),          # sum 494
]


def _plan(N_l):
    """Return (Cs, slot_map): slot capacities and slot_map[core][s] =
    (l, col_start, cnt) or None."""
    order_l = np.argsort(-np.asarray(N_l), kind="stable")
    items = [int(N_l[i]) for i in order_l if N_l[i] > 0]
    kept_l = [int(i) for i in order_l if N_l[i] > 0]
    if not items:
        return [2], [[None] for _ in range(NCORES)]

    best = None  # (sumC, Cs, picks)
    for Cs in _CANDIDATE_CS:
        if best is not None and sum(Cs) >= best[0]:
            continue
        picks = _fit(Cs, items)
        if picks is not None:
            best = (sum(Cs), list(Cs), picks)

    if best is None:
        # guaranteed-feasible fallback: biggest items unsplit in slot 0,
        # the rest in slot 1
        c1 = items[0]
        c2 = items[8] if len(items) > 8 else 0
        Cs = [c1, c2] if c2 else [c1]
        picks = _fit(Cs, items, max_waste=10**9)
        assert picks is not None
        best = (sum(Cs), Cs, picks)

    _, Cs, picks = best
    S = len(Cs)
    slot_pieces = [[] for _ in range(S)]
    for idx, l in enumerate(kept_l):
        sz = items[idx]
        pos = 0
        for p in range(S):
            for _ in range(picks[idx][p]):
                cnt = min(Cs[p], sz - pos)
                if cnt <= 0:
                    continue
                slot_pieces[p].append((l, pos, cnt))
                pos += cnt
        assert pos >= sz, f"l={l} not covered: {pos}/{sz}"

    slot_map = [[None] * S for _ in range(NCORES)]
    for p in range(S):
        assert len(slot_pieces[p]) <= NCORES, (p, slot_pieces[p])
        for c, piece in enumerate(slot_pieces[p]):
            slot_map[c][p] = piece
    return Cs, slot_map


# ---------------------------------------------------------------------------
# device program
# ---------------------------------------------------------------------------

def _chunks(total):
    n = -(-total // CHMAX)
    base = total // n
    rem = total - base * n
    return [base + (1 if i < rem else 0) for i in range(n)]


def _proc_order(Cs):
    desc = list(np.argsort(Cs)[::-1])
    if len(desc) >= 3:
        # mid-size slot first (small enough that its per-k input stream
        # beats the k-outer sweep), then the largest (second k-outer slot,
        # streamed per-k behind slot0's compute), then the rest descending
        return [desc[2], desc[0], desc[1]] + desc[3:]
    return desc


def _slot_chunks(Cs, s):
    F = B * Cs[s]
    proc = _proc_order(Cs)
    if s in proc[:2]:
        # k-outer slots: big first chunk so early k-rounds are slow
        # enough for the per-k input DMAs; all chunks >=128 cols so
        # ldweights stays hidden under the stream
        first = min(F, 448 if s == proc[0] else CHMAX)
        if F - first >= 128:
            return [first] + _chunks(F - first)
        return [F]
    return _chunks(F)


def _build_program(Cs):
    """Inputs per slot s: x{s} (128, KC*8*C_s) f16, w{s} (128, KC*D) f16.
    Outputs per (slot, chunk): y{s}_{i} (128, MC*ch) f16."""
    nc = bacc.Bacc("TRN2", target_bir_lowering=False, debug=False)
    S = len(Cs)
    # processing order: DESCENDING by capacity.  The first slot runs
    # k-outer with per-k-chunk input DMAs, so it only waits on k0's
    # ~0.6MB and each ~1.4-2.9us k-round covers the next k-chunk's
    # delivery; its long compute then buys DMA time for all later
    # slots' (W-heavy) inputs.  Small slots last also makes the output
    # tail tiny.  (Measured: small-first stalls the PE 17-20us because
    # small slots' 2MB weight loads outpace their compute.)
    proc = _proc_order(Cs)

    def slot_chunks(s):
        return _slot_chunks(Cs, s)

    xs, ws, ys = {}, {}, {}
    for s, C in enumerate(Cs):
        xs[s] = nc.dram_tensor(f"x{s}", (128, KC * B * C), F16, kind="ExternalInput")
        ws[s] = nc.dram_tensor(f"w{s}", (128, KC * D), F16, kind="ExternalInput")
        for i, ch in enumerate(slot_chunks(s)):
            ys[s, i] = nc.dram_tensor(
                f"y{s}_{i}", (128, MC * ch), F16, kind="ExternalOutput"
            )

    with tile.TileContext(nc) as tc, ExitStack() as ctx:
        wpool = ctx.enter_context(tc.tile_pool(name="w", bufs=1))
        xpool = ctx.enter_context(tc.tile_pool(name="x", bufs=1))
        opool = ctx.enter_context(tc.tile_pool(name="o", bufs=3))
        ppool = ctx.enter_context(tc.tile_pool(name="ps", bufs=8, space="PSUM"))

        wt, xt = {}, {}
        for s in range(S):
            C = Cs[s]
            wt[s] = wpool.tile([128, KC * D], F16, tag=f"w{s}", name=f"w{s}")
            xt[s] = xpool.tile([128, KC * B * C], F16, tag=f"x{s}", name=f"x{s}")

        # input DMAs in processing order, W on scalar || X on sync (the two
        # HWDGE rings).  Each ring runs one DMA at a time with ~2us fixed
        # completion latency (measured), so the critical path wants FEW
        # DMAs; slot proc[0] is split 3 ways ([k0], [k1:4], [k4:8]) so the
        # first matmul waits only on ~0.6MB while later k-chunks amortize
        # the latency over MB-sized transfers.
        # PE warmup: dummy matmuls from a memset tile start the DVFS
        # pstate ramp (~3us at half clock) before real inputs land, so
        # real matmuls run at full clock from the start.  Sized to end
        # right as the first real inputs become ready (~11us).
        zt = wpool.tile([128, CHMAX], F16, tag="z", name="zt")
        nc.vector.memset(zt[:], 0.0)
        for i in range(7):
            zp = ppool.tile([128, CHMAX], F32, tag="ps", name=f"warm{i}")
            nc.tensor.matmul(zp[:], zt[:, :128], zt[:], start=True, stop=True)

        for j, s in enumerate(proc[:2]):
            F = B * Cs[s]
            for i, (k0, k1) in enumerate(((0, 1), (1, 2), (2, 4), (4, KC))):
                qa, qb = (
                    (nc.scalar, nc.sync) if (i + j) % 2 == 0 else (nc.sync, nc.scalar)
                )
                qa.dma_start(
                    wt[s][:, k0 * D : k1 * D], ws[s].ap()[:, k0 * D : k1 * D]
                )
                qb.dma_start(
                    xt[s][:, k0 * F : k1 * F], xs[s].ap()[:, k0 * F : k1 * F]
                )
        for s in proc[2:]:
            nc.scalar.dma_start(wt[s][:], ws[s].ap()[:])
            nc.sync.dma_start(xt[s][:], xs[s].ap()[:])

        def emit_slot(s, k_outer, fine_tail):
            C = Cs[s]
            F = B * C
            chs = slot_chunks(s)
            off = 0
            for i, ch in enumerate(chs):
                ot = opool.tile([128, MC, ch], F16, tag="o", name=f"o{s}_{i}")
                if k_outer:
                    pss = [
                        ppool.tile([128, ch], F32, tag="ps", name=f"ps{s}_{i}_{m}")
                        for m in range(MC)
                    ]
                    for k in range(KC):
                        for m in range(MC):
                            nc.tensor.matmul(
                                pss[m][:],
                                wt[s][:, k * D + m * 128 : k * D + (m + 1) * 128],
                                xt[s][:, k * F + off : k * F + off + ch],
                                start=(k == 0),
                                stop=(k == KC - 1),
                            )
                    for m in range(MC):
                        nc.vector.tensor_copy(ot[:, m], pss[m][:])
                else:
                    for m in range(MC):
                        ps = ppool.tile([128, ch], F32, tag="ps", name=f"ps{s}_{i}_{m}")
                        for k in range(KC):
                            nc.tensor.matmul(
                                ps[:],
                                wt[s][:, k * D + m * 128 : k * D + (m + 1) * 128],
                                xt[s][:, k * F + off : k * F + off + ch],
                                start=(k == 0),
                                stop=(k == KC - 1),
                            )
                        nc.vector.tensor_copy(ot[:, m], ps[:])
                y = ys[s, i]
                if fine_tail and i == len(chs) - 1:
                    # drain the last chunk via the two idle HWDGE rings
                    # as 2-m-slice DMAs: each piece issues right after
                    # its PSUM copies, pipelining the drain with compute
                    h = 2
                    for mp in range(MC // h):
                        q = nc.sync if mp % 2 == 0 else nc.scalar
                        q.dma_start(
                            y.ap()[:, mp * h * ch : (mp + 1) * h * ch].rearrange(
                                "p (m c) -> p m c", m=h
                            ),
                            ot[:, mp * h : (mp + 1) * h],
                        )
                else:
                    nc.gpsimd.dma_start(
                        y.ap().rearrange("p (m c) -> p m c", m=MC), ot[:]
                    )
                off += ch

        for j, s in enumerate(proc):
            emit_slot(s, k_outer=(j < 2), fine_tail=(j == len(proc) - 1))

    nc.compile()
    return nc


# ---------------------------------------------------------------------------
# host wrapper
# ---------------------------------------------------------------------------

def _segment_structure(idx, T):
    t = np.arange(T)
    seg = np.searchsorted(idx, t, side="left")
    valid = seg < N
    segc = np.clip(seg, 0, N - 1)
    start = np.where(segc > 0, idx[np.maximum(segc - 1, 0)] + 1, 0)
    lidx = np.minimum(t - start, L - 1).astype(np.int64)
    lens = np.bincount(segc[valid], minlength=N)
    return t, seg, valid, segc, lidx, lens


def _install_ntff_hook():
    """Profiling-only: register the axon NTFF profile hook (dev use)."""
    import sys
    import types

    try:
        import antenv

        if "antenv.axon_hooks" not in sys.modules:
            mod = types.ModuleType("antenv.axon_hooks")
            holder = [None]
            mod.set_axon_ntff_profile_hook = lambda h: holder.__setitem__(0, h)
            mod.get_axon_ntff_profile_hook = lambda: holder[0]
            sys.modules["antenv.axon_hooks"] = mod
            antenv.axon_hooks = mod
            from trn_agent_boot.trn_boot import _ntff_profile_via_ctypes

            mod.set_axon_ntff_profile_hook(
                _ntff_profile_via_ctypes("/opt/axon/libaxon_pjrt.so")
            )
    except Exception as e:
        print(f"NTFF hook install failed: {e}")


def kernel(pooled_vectors, W, pooling_indices, target_length, _trace=False):
    pooled = np.asarray(pooled_vectors, dtype=np.float32)
    Wf = np.asarray(W, dtype=np.float32)
    idx = np.asarray(pooling_indices).astype(np.int64)
    T = int(np.asarray(target_length))

    t, seg, valid, segc, lidx, lens = _segment_structure(idx, T)

    order = np.argsort(-lens, kind="stable")
    rank_of_seg = np.empty(N, dtype=np.int64)
    rank_of_seg[order] = np.arange(N)
    N_l = (lens[None, :] > np.arange(L)[:, None]).sum(axis=1)

    Cs, slot_map = _plan(N_l)
    S = len(Cs)

    nc = _build_program(Cs)

    # host-side gathered inputs, fp16
    # Xg: (D, B, N) with columns sorted by segment rank
    Xg = np.ascontiguousarray(pooled.transpose(2, 0, 1)[:, :, order]).astype(
        np.float16
    )
    Xg_k = Xg.reshape(KC, 128, B, N)
    # Wt16[l]: (128, KC*D) with row p, block k = W[l][:, k*128+p]
    Wt16 = np.ascontiguousarray(
        Wf.transpose(2, 0, 1).reshape(KC, 128, L, D).transpose(1, 2, 0, 3)
    ).astype(np.float16)  # (128, L, KC, D)

    in_maps = []
    for c in range(NCORES):
        im = {}
        for s in range(S):
            C = Cs[s]
            xp = np.zeros((128, KC, B, C), dtype=np.float16)
            wp = np.zeros((128, KC * D), dtype=np.float16)
            piece = slot_map[c][s]
            if piece is not None:
                l, c0, cnt = piece
                xp[:, :, :, :cnt] = Xg_k[:, :, :, c0 : c0 + cnt].transpose(1, 0, 2, 3)
                wp[:] = Wt16[:, l].reshape(128, KC * D)
            im[f"x{s}"] = np.ascontiguousarray(xp.reshape(128, KC * B * C))
            im[f"w{s}"] = wp
        in_maps.append(im)

    kwargs = {}
    if _trace:
        _install_ntff_hook()
        kwargs = dict(trace=True)
    res = run_bass_kernel_spmd(nc, in_maps, core_ids=list(range(NCORES)), **kwargs)
    results = res.results

    # per-(l, col-rank) -> (core, slot, j) maps
    maxN = int(N_l.max()) if L else 0
    core_of = np.full((L, max(maxN, 1)), -1, dtype=np.int32)
    slot_of = np.zeros((L, max(maxN, 1)), dtype=np.int32)
    j_of = np.zeros((L, max(maxN, 1)), dtype=np.int32)
    for c in range(NCORES):
        for s in range(S):
            piece = slot_map[c][s]
            if piece is None:
                continue
            l, c0, cnt = piece
            core_of[l, c0 : c0 + cnt] = c
            slot_of[l, c0 : c0 + cnt] = s
            j_of[l, c0 : c0 + cnt] = np.arange(cnt)

    Dout = Wf.shape[1]
    out = np.zeros((B, T, Dout), dtype=np.float32)
    tv = t[valid]
    l_t = lidx[valid]
    r_t = rank_of_seg[segc[valid]]
    ct = core_of[l_t, r_t]
    st = slot_of[l_t, r_t]
    jt = j_of[l_t, r_t]
    assert (ct >= 0).all(), "uncovered (l, col) in assignment"

    for s in range(S):
        sel = st == s
        if not sel.any():
            continue
        C = Cs[s]
        chs = _slot_chunks(Cs, s)
        # per core: concat chunks -> (B*C, 1024), then (B, C, 1024)
        Ys = np.empty((NCORES, B, C, Dout), dtype=np.float16)
        for c in range(NCORES):
            parts = []
            for i, ch in enumerate(chs):
                a = results[c][f"y{s}_{i}"].reshape(128, MC, ch)
                parts.append(a.transpose(2, 1, 0).reshape(ch, Dout))
            Ys[c] = np.concatenate(parts, axis=0).reshape(B, C, Dout)
        out[:, tv[sel], :] = Ys[ct[sel], :, jt[sel]].transpose(1, 0, 2).astype(
            np.float32
        )

    if _trace:
        kernel._last_exec_time_ns = res.exec_time_ns
        kernel._last_results = res
    return out
